# revision 1
# baseline (speedup 1.0000x reference)
"""BEVFeatureAggregation Trainium2 kernel.

Math: out[b,n,o] = inst[b,n,o] + b_proj[o]
                 + sum_c W_proj[o,c] * bilinear_sample(bev_map[b], anchor[b,n])[c]

Strategy (8 NeuronCores, core = batch*2 + anchor-half, 5000 anchors each):
  * anchors concentrate in a tiny window of the 200x400 BEV map; the host
    computes the bounding box (R rows x K cols) of all touched bilinear
    corners and ships only that subregion (C x R*K, zero-padded) per core.
  * the device projects the subregion first:  S'[px,o] = sum_c sub[c,px] *
    W_proj[o,c]  (small fp32 matmuls producing per-row-pair tiles), so
    sampling directly produces output features.
  * the host sorts anchors by their bilinear row y0 (un-permuting on the
    way out).  All 4 corners of an anchor with row y0 live in the 2*K-pixel
    window [y0*K, y0*K+2K) of the row-major subregion, so each sorted
    group's sampling is a dense matmul with contraction only over that
    window (<=128 typically) instead of the whole R*K bbox:
        out_T[o, n] = sum_px S'pair[px, o] * wb[px, n]
    wb (<=128 x NSLOT) holds the 4 bilinear corner weights per column.
  * weights and S' are split hi/lo in bf16 (hi+lo carries ~2^-18 relative
    precision); three bf16 matmul passes (Shi*Whi + Shi*Wlo + Slo*Whi)
    give fp32-grade accuracy at 1 PE cycle/row instead of fp32's 4.
  * epilogue fuses the residual add (instance_feature + b_proj, transposed
    and permuted on host) while copying PSUM out; host transposes /
    un-permutes the (C, NSLOT) result.
  * dummy matmuls keep the PE HAM clock warm while the initial DMAs land.

All 8 cores run one SPMD program whose loop structure (subtile layout) is
the per-row max across cores; it is rebuilt (and the NEFF recompiled) when
that structure changes, and cached for repeated calls with the same
structure.
"""

import numpy as np
import ml_dtypes

import concourse.bass as bass
import concourse.mybir as mybir
import concourse.tile as tile
from concourse.bass_utils import run_bass_kernel_spmd

# ---------------------------------------------------------------- constants
XMIN, XMAX, YMIN, YMAX = -80.0, 120.0, -40.0, 40.0
EPS = 1e-6
B, N, C, H, W = 4, 10000, 256, 200, 400
NCORES = 8
NPC = B * N // NCORES          # anchors per core
RK_MAX = 4096                  # bbox cap; beyond this fall back to host
SUBTILE = 512                  # max psum free width
WARMUP_MM = 10                 # dummy matmuls to keep the PE HAM-warm
BRIDGE_MM = 6                  # dummy matmuls between phase 1 and 2
DMA_PIECES = 4                 # split big loads so they complete in order
F32 = mybir.dt.float32
BF16 = mybir.dt.bfloat16
NPBF16 = ml_dtypes.bfloat16

TRACE = False                  # set by test harness for profiling runs
LAST_RESULT = None             # BassKernelResults of the last device run

# --------------------------------------------------- walrus 1-wait workaround
# This container's walrus rejects >1 sem wait per instruction ("Too many
# sync wait commands").  Spread extra waits onto same-engine NoOps.

_MAXW = 1
_ctr = [0]


def _patched_drain_and_barrier(self, tick_clock, wait_clock):
    nc = self.nc
    probe = nc.sync.nop(hint="drain_wait_spread", nofuse=True)
    wait_clock.add_sem_waits(
        probe.ins, tile.ScopedClock({None: tick_clock.global_clock})
    )
    waits = list(probe.ins.sync_info.on_wait or [])
    if len(waits) > _MAXW:
        probe.ins.sync_info.on_wait = waits[:_MAXW]
        rest = waits[_MAXW:]
        while rest:
            chunk, rest = rest[:_MAXW], rest[_MAXW:]
            nxt = nc.sync.nop(hint="drain_wait_spread", nofuse=True)
            if nxt.ins.sync_info is None:
                nxt.ins.sync_info = mybir.SyncInfo(on_wait=chunk, on_update=[])
            else:
                nxt.ins.sync_info.on_wait = chunk
    nc.sync.drain()
    # One barrier (not two) before the semaphore cleanup; nothing runs after
    # the cleanup, so the trailing barrier of the stock tail is dropped.
    nc.all_engine_barrier()
    assert self.sems is not None
    popped = nc._tile_sem_poison_stack.pop()
    assert popped is self._sem_poison
    nc.clear_and_free_semaphores(list(self.sems.allocated().values()))


tile.TileContext._drain_and_barrier = _patched_drain_and_barrier


def _split_multiwait(nc):
    for f in nc.m.functions:
        for b in f.blocks:
            insts = list(b.instructions)
            out = []
            changed = False
            for inst in insts:
                si = inst.sync_info
                waits = list(si.on_wait) if (si and si.on_wait) else []
                if len(waits) > _MAXW:
                    changed = True
                    extra, keep = waits[:-_MAXW], waits[-_MAXW:]
                    si.on_wait = keep
                    inst.sync_info = si
                    for w in extra:
                        _ctr[0] += 1
                        nop = mybir.InstNoOp(
                            name=f"wsplit_{_ctr[0]}", ins=[], outs=[]
                        )
                        nop.engine = inst.engine
                        nop.sync_info = mybir.SyncInfo(on_wait=[w], on_update=[])
                        out.append(nop)
                out.append(inst)
            if changed:
                cur = b.instructions
                while len(cur):
                    cur.pop()
                for inst in out:
                    b.add_instruction(inst)


# ------------------------------------------------------------ device program
# structure = (rkp, Kw, ws, kch, n_pairs, nslot, subtiles); subtiles is a
# tuple of (pair_idx, col_offset, width).
_programs = {}


def _build_program(structure):
    rkp, Kw, ws, kch, n_pairs, nslot, subtiles = structure
    nc = bass.Bass()
    bevh = nc.declare_dram_parameter("bev_hi", [C, rkp], BF16, isOutput=False)
    bevl = nc.declare_dram_parameter("bev_lo", [C, rkp], BF16, isOutput=False)
    wpth = nc.declare_dram_parameter("wpt_hi", [C, C], BF16, isOutput=False)
    wptl = nc.declare_dram_parameter("wpt_lo", [C, C], BF16, isOutput=False)
    wbh = nc.declare_dram_parameter("wb_hi", [kch * 128, nslot], BF16,
                                    isOutput=False)
    wbl = nc.declare_dram_parameter("wb_lo", [kch * 128, nslot], BF16,
                                    isOutput=False)
    ins = nc.declare_dram_parameter("instb_t", [C, nslot], F32, isOutput=False)
    out = nc.declare_dram_parameter("out_t", [C, nslot], F32, isOutput=True)

    with tile.TileContext(nc) as tc:
        with (
            tc.tile_pool(name="const", bufs=1) as constp,
            tc.tile_pool(name="io", bufs=4) as iop,
            tc.tile_pool(name="ob", bufs=1) as obp,
            tc.tile_pool(name="ps", bufs=8, space="PSUM") as psp,
        ):
            # ---- PE warmup first: dummy matmuls on an uninitialized tile
            # (result never read) keep the HAM clock hot while DMAs land.
            wu = constp.tile([128, 512], BF16, tag="warm", name="warm")
            nc.gpsimd.memset(wu[:], 0.0)
            wups = psp.tile([128, SUBTILE], F32, tag="ps", name="wups")
            for _ in range(WARMUP_MM):
                nc.tensor.matmul(wups[:], lhsT=wu[:, 0:128], rhs=wu[:],
                                 start=True, stop=True)

            # ---- input DMAs, one per array: the SP HWDGE ring is FIFO and
            # each dma_start costs ~0.6us of SP issue time, so few + ordered
            # is best (bev/wpt gate phase 1, wb gates phase 2, inst only
            # gates the epilogue adds).
            bev_sb = {}
            for hl, src in (("h", bevh), ("l", bevl)):
                for cc in range(2):
                    t = constp.tile([128, rkp], BF16, tag=f"bev{hl}{cc}",
                                    name=f"bev{hl}{cc}")
                    nc.sync.dma_start(t[:], src[cc * 128:(cc + 1) * 128, :])
                    bev_sb[hl, cc] = t
            wpt_sb = {}
            for hl, src in (("h", wpth), ("l", wptl)):
                for cc in range(2):
                    t = constp.tile([128, C], BF16, tag=f"wpt{hl}{cc}",
                                    name=f"wpt{hl}{cc}")
                    nc.sync.dma_start(t[:], src[cc * 128:(cc + 1) * 128, :])
                    wpt_sb[hl, cc] = t
            # wb hi/lo interleaved in column pieces: the first subtiles only
            # need the leading columns of BOTH hi and lo, so those arrive
            # first and phase 2 starts earlier.
            wsplit = min(2048, nslot)
            wb_sb = [[None] * kch for _ in range(2)]
            for ci, src in enumerate((wbh, wbl)):
                for ch in range(kch):
                    wb_sb[ci][ch] = constp.tile(
                        [128, nslot], BF16, tag=f"wb{ci}_{ch}",
                        name=f"wb{ci}_{ch}")
            for s0, s1 in ((0, wsplit), (wsplit, nslot)):
                if s0 >= s1:
                    continue
                for ci, src in enumerate((wbh, wbl)):
                    for ch in range(kch):
                        nc.sync.dma_start(
                            wb_sb[ci][ch][:, s0:s1],
                            src[ch * 128:(ch + 1) * 128, s0:s1])
            inst_sb = []
            for oc in range(2):
                t = constp.tile([128, nslot], F32, tag=f"instb{oc}",
                                name=f"instb{oc}")
                inst_sb.append(t)
            for s0, s1 in ((0, wsplit), (wsplit, nslot)):
                if s0 >= s1:
                    continue
                for oc in range(2):
                    nc.sync.dma_start(
                        inst_sb[oc][:, s0:s1],
                        ins[oc * 128:(oc + 1) * 128, s0:s1])

            # ---- phase 1: project row-pair windows, split hi/lo
            # pair r covers subregion pixels [r*Kw, r*Kw + ws)
            sp_hi, sp_lo = [], []
            for r in range(n_pairs):
                his, los = [], []
                for ch in range(kch):
                    p0 = r * Kw + ch * 128
                    pw = max(0, min(128, ws - ch * 128, rkp - p0))
                    if pw == 0:
                        his.append((None, 0))
                        los.append((None, 0))
                        continue
                    ps = psp.tile([128, SUBTILE], F32, tag="ps",
                                  name=f"ps1_{r}_{ch}")
                    p1passes = [("h", "h"), ("h", "l"), ("l", "h")]
                    for pi, (bhl, whl) in enumerate(p1passes):
                        for cc in range(2):
                            nc.tensor.matmul(
                                ps[0:pw, 0:C],
                                lhsT=bev_sb[bhl, cc][:, p0:p0 + pw],
                                rhs=wpt_sb[whl, cc][:],
                                start=(pi == 0 and cc == 0),
                                stop=(pi == 2 and cc == 1),
                            )
                    hi = constp.tile([128, C], BF16, tag=f"sph{r}_{ch}",
                                     name=f"sph{r}_{ch}")
                    lo = constp.tile([128, C], BF16, tag=f"spl{r}_{ch}",
                                     name=f"spl{r}_{ch}")
                    nc.vector.tensor_copy(hi[0:pw, 0:C], ps[0:pw, 0:C])
                    tmp = iop.tile([128, C], F32, tag="split_tmp",
                                   name="split_tmp")
                    nc.any.tensor_copy(tmp[0:pw, :], hi[0:pw, 0:C])
                    nc.vector.tensor_sub(lo[0:pw, 0:C], ps[0:pw, 0:C],
                                         tmp[0:pw, :])
                    his.append((hi, pw))
                    los.append((lo, pw))
                sp_hi.append(his)
                sp_lo.append(los)

            # bridge dummies: keep the PE busy while wb/inst DMAs land
            for _ in range(BRIDGE_MM):
                nc.tensor.matmul(wups[:], lhsT=wu[:, 0:128], rhs=wu[:],
                                 start=True, stop=True)

            # ---- phase 2: sampling matmuls; psum is copied out immediately
            # (frees the bank without waiting for inst), the residual is
            # added in place once inst lands, and output blocks go out on
            # the scalar HWDGE ring (parallel with the SP input ring).
            blocks = []            # (b0, bw, [subtiles])
            for (r, c0, tw) in subtiles:
                if blocks and (c0 + tw - blocks[-1][0]) <= 1344:
                    blocks[-1][2].append((r, c0, tw))
                    blocks[-1][1] = c0 + tw - blocks[-1][0]
                else:
                    blocks.append([c0, tw, [(r, c0, tw)]])
            # emission is software-pipelined: block b's copies go out, then
            # block b-ADD_LAG's residual adds + DMA.  By the time the DVE
            # FIFO reaches an add, its inst half has landed, so the adds
            # never stall the copies that recycle psum banks.
            def add_lag(wi):
                return 4 if wi < len(work) - 4 else 1
            work = []            # (oc, ob, b0, bw, sts)
            for oc in range(2):
                for b0, bw, sts in blocks:
                    ob = obp.tile([128, 1344], F32, tag=f"outblk_{oc}_{b0}",
                                  name=f"outblk_{oc}_{b0}")
                    work.append((oc, ob, b0, bw, sts))

            direct_from = max(0, len(work) - 4)

            def emit_adds(wi):
                oc, ob, b0, bw, sts = work[wi]
                if wi < direct_from:   # direct items fused the add already
                    for (r, c0, tw) in sts:
                        lc = c0 - b0
                        nc.vector.tensor_add(
                            ob[:, lc:lc + tw], ob[:, lc:lc + tw],
                            inst_sb[oc][:, c0:c0 + tw],
                        )
                nc.scalar.dma_start(
                    out[oc * 128:(oc + 1) * 128, b0:b0 + bw], ob[:, 0:bw]
                )

            next_add = 0
            sti = 0
            for wi, (oc, ob, b0, bw, sts) in enumerate(work):
                for (r, c0, tw) in sts:
                    sti += 1
                    ps = psp.tile([128, SUBTILE], F32, tag="ps",
                                  name=f"ps2_{oc}_{c0}")
                    mms = []
                    for sp, wbi in ((sp_hi, 0), (sp_hi, 1), (sp_lo, 0)):
                        for ch in range(kch):
                            t, pw = sp[r][ch]
                            if pw:
                                mms.append((t, pw, wbi, ch))
                    for i, (t, pw, wbi, ch) in enumerate(mms):
                        nc.tensor.matmul(
                            ps[:, 0:tw],
                            lhsT=t[0:pw, oc * 128:(oc + 1) * 128],
                            rhs=wb_sb[wbi][ch][0:pw, c0:c0 + tw],
                            start=(i == 0),
                            stop=(i == len(mms) - 1),
                        )
                    lc = c0 - b0
                    if wi >= direct_from:
                        # inst has landed by now: fused psum+inst add frees
                        # the bank in one op
                        nc.vector.tensor_add(
                            ob[:, lc:lc + tw], ps[:, 0:tw],
                            inst_sb[oc][:, c0:c0 + tw],
                        )
                    elif sti % 2:
                        nc.vector.tensor_copy(ob[:, lc:lc + tw], ps[:, 0:tw])
                    else:
                        nc.scalar.copy(ob[:, lc:lc + tw], ps[:, 0:tw])
                while next_add <= wi - add_lag(wi):
                    emit_adds(next_add)
                    next_add += 1
            while next_add < len(work):
                emit_adds(next_add)
                next_add += 1

    return nc


def _get_program(structure):
    if structure not in _programs:
        nc = _build_program(structure)
        _split_multiwait(nc)
        nc._wsplit_done = True
        _programs[structure] = nc
    return _programs[structure]


# -------------------------------------------------------------- host prep
def _corners(anchor_bn):
    f = np.float32
    ax = anchor_bn[:, 0].astype(f)
    ay = anchor_bn[:, 1].astype(f)
    gx = (ax - f(XMIN)) / f(XMAX - XMIN + EPS) * f(2.0) - f(1.0)
    gy = (ay - f(YMIN)) / f(YMAX - YMIN + EPS) * f(2.0) - f(1.0)
    # module stacks [grid_y, grid_x]: width coord <- gy, height coord <- gx
    ix = (gy + f(1.0)) * f(0.5) * f(W - 1)
    iy = (gx + f(1.0)) * f(0.5) * f(H - 1)
    x0 = np.floor(ix)
    y0 = np.floor(iy)
    x1 = x0 + f(1.0)
    y1 = y0 + f(1.0)
    wx1 = ix - x0
    wx0 = f(1.0) - wx1
    wy1 = iy - y0
    wy0 = f(1.0) - wy1
    out = []
    for xc, yc, w in ((x0, y0, wx0 * wy0), (x1, y0, wx1 * wy0),
                      (x0, y1, wx0 * wy1), (x1, y1, wx1 * wy1)):
        valid = (xc >= 0) & (xc <= W - 1) & (yc >= 0) & (yc <= H - 1)
        xi = np.clip(xc, 0, W - 1).astype(np.int64)
        yi = np.clip(yc, 0, H - 1).astype(np.int64)
        out.append((xi, yi, valid, (w * valid.astype(f)).astype(f)))
    return out, y0


def _host_fallback(instance_feature, anchor, bev_map, W_proj, b_proj):
    """Exact numpy computation; only for pathological inputs whose bbox
    exceeds RK_MAX."""
    f = np.float32
    out = np.empty((B, N, C), f)
    for b in range(B):
        corners, _ = _corners(anchor[b])
        acc = np.zeros((N, C), f)
        fm = bev_map[b].reshape(C, H * W)
        for xi, yi, valid, w in corners:
            g = fm[:, yi * W + xi].T
            acc += g * w[:, None]
        out[b] = acc @ W_proj.T.astype(f) + b_proj.astype(f)
    return out + instance_feature.astype(f)


# ------------------------------------------------------------------- kernel
def kernel(instance_feature, anchor, anchor_embed, bev_map, W_proj, b_proj):
    global LAST_RESULT
    f = np.float32
    instance_feature = np.asarray(instance_feature)
    anchor = np.asarray(anchor)
    bev_map = np.asarray(bev_map)
    W_proj = np.asarray(W_proj)
    b_proj = np.asarray(b_proj)

    instb = instance_feature.astype(f) + b_proj.astype(f)[None, None, :]

    # ---- pass 1: per-core corner geometry
    cores = []
    for core in range(NCORES):
        b, half = core // 2, core % 2
        sl = slice(half * NPC, (half + 1) * NPC)
        corners, y0f = _corners(anchor[b, sl])
        vx = np.concatenate([np.where(v, xi, -1) for xi, yi, v, w in corners])
        vy = np.concatenate([np.where(v, yi, -1) for xi, yi, v, w in corners])
        m = vx >= 0
        if m.any():
            xmin, xmax = int(vx[m].min()), int(vx[m].max())
            ymin, ymax = int(vy[m].min()), int(vy[m].max())
        else:
            xmin = xmax = ymin = ymax = 0
        R, K = ymax - ymin + 1, xmax - xmin + 1
        if R * K > RK_MAX:
            return _host_fallback(instance_feature, anchor, bev_map,
                                  W_proj, b_proj)
        cores.append((corners, y0f, xmin, ymin, R, K))

    # ---- unified structure
    Kw = max(c[5] for c in cores)
    n_pairs = max(max(c[4] - 1, 1) for c in cores)
    ws = 2 * Kw
    kch = -(-ws // 128)
    rkp = 128 * -(-max((n_pairs - 1) * Kw + ws,
                       max(c[4] * Kw for c in cores)) // 128)
    if rkp > RK_MAX:
        return _host_fallback(instance_feature, anchor, bev_map,
                              W_proj, b_proj)

    y0ps = []
    counts = np.zeros((NCORES, n_pairs), np.int64)
    for core, (corners, y0f, xmin, ymin, R, K) in enumerate(cores):
        y0p = np.clip(y0f.astype(np.int64) - ymin, 0, max(R - 2, 0))
        y0p = np.minimum(y0p, n_pairs - 1)
        y0ps.append(y0p)
        counts[core] = np.bincount(y0p, minlength=n_pairs)
    cap = counts.max(axis=0)

    subtiles = []
    c0 = 0
    for r in range(n_pairs):
        left = int(cap[r])
        while left > 0:
            tw = min(SUBTILE, left)
            subtiles.append((r, c0, tw))
            c0 += tw
            left -= tw
    nslot = c0
    structure = (rkp, Kw, ws, kch, n_pairs, nslot, tuple(subtiles))

    # ---- pass 2: per-core arrays against the unified layout
    row_base = {}
    base = 0
    for r in range(n_pairs):
        row_base[r] = base
        base += int(cap[r])

    maps, perms = [], []
    wpt = np.ascontiguousarray(W_proj.astype(f).T)
    wpt_hi = wpt.astype(NPBF16)
    wpt_lo = (wpt - wpt_hi.astype(f)).astype(NPBF16)
    for core, (corners, y0f, xmin, ymin, R, K) in enumerate(cores):
        b, half = core // 2, core % 2
        sl = slice(half * NPC, (half + 1) * NPC)
        y0p = y0ps[core]
        # stable sort by pair row; columns are packed at each row's base
        order = np.argsort(y0p, kind="stable")
        cnt = counts[core]
        col_of = np.empty(NPC, np.int64)
        start = 0
        for r in range(n_pairs):
            end = start + int(cnt[r])
            col_of[order[start:end]] = row_base[r] + np.arange(end - start)
            start = end

        bev_sub = np.zeros((C, rkp), f)
        ke = min(xmin + Kw, W)
        bev_rows = bev_map[b][:, ymin:ymin + R, xmin:ke].astype(f)
        tmp = np.zeros((C, R, Kw), f)
        tmp[:, :, :ke - xmin] = bev_rows
        bev_sub[:, :R * Kw] = tmp.reshape(C, R * Kw)
        bev_hi = bev_sub.astype(NPBF16)
        bev_lo = (bev_sub - bev_hi.astype(f)).astype(NPBF16)

        wb_hi = np.zeros((kch * 128, nslot), NPBF16)
        wb_lo = np.zeros((kch * 128, nslot), NPBF16)
        for xi, yi, valid, wgt in corners:
            px = (yi - ymin - y0p) * Kw + (xi - xmin)
            col = col_of[valid]
            pxv = px[valid]
            hi = wgt[valid].astype(NPBF16)
            lo = (wgt[valid] - hi.astype(f)).astype(NPBF16)
            wb_hi[pxv, col] = hi
            wb_lo[pxv, col] = lo

        instb_t = np.zeros((C, nslot), f)
        instb_t[:, col_of] = instb[b, sl].T

        maps.append({
            "bev_hi": bev_hi,
            "bev_lo": bev_lo,
            "wb_hi": wb_hi,
            "wb_lo": wb_lo,
            "instb_t": instb_t,
            "wpt_hi": wpt_hi,
            "wpt_lo": wpt_lo,
        })
        perms.append(col_of)

    nc = _get_program(structure)
    res = run_bass_kernel_spmd(nc, maps, list(range(NCORES)), trace=TRACE)
    LAST_RESULT = res

    out = np.empty((B, N, C), f)
    for core in range(NCORES):
        b, half = core // 2, core % 2
        sl = slice(half * NPC, (half + 1) * NPC)
        o = res.results[core]["out_t"]
        out[b, sl] = o[:, perms[core]].T
    return out



# revision 2
# speedup vs baseline: 1.1758x; 1.1758x over previous
"""BEVFeatureAggregation Trainium2 kernel.

Math: out[b,n,o] = inst[b,n,o] + b_proj[o]
                 + sum_c W_proj[o,c] * bilinear_sample(bev_map[b], anchor[b,n])[c]

Strategy (8 NeuronCores, core = batch*2 + anchor-half, 5000 anchors each):
  * anchors concentrate in a tiny window of the 200x400 BEV map; the host
    computes the bounding box (R rows x K cols) of all touched bilinear
    corners and ships only that subregion (C x R*K, zero-padded) per core.
  * the device projects the subregion first:  S'[px,o] = sum_c sub[c,px] *
    W_proj[o,c], so sampling directly produces output features.
  * the host sorts anchors by their bilinear row y0 (un-permuting on the
    way out).  All 4 corners of an anchor with row y0 live in the 2*K-pixel
    window [y0*K, y0*K+2K) of the row-major subregion, so each sorted
    group's sampling is a dense matmul with contraction only over that
    window (<=128 typically) instead of the whole R*K bbox:
        out_T[o, n] = sum_px S'pair[px, o] * wb[px, n]
    wb (<=128 x NSLOT) holds the 4 bilinear corner weights per column.
  * everything is single bf16 (tolerance is 2e-2; measured pipeline error
    ~5e-3), which halves HBM traffic vs fp32 and needs 1 matmul pass.
  * the residual (instance_feature + b_proj, transposed and permuted on
    host, bf16) is added by the DMA engine: an SWDGE accumulating load
    (accum_op=add) lands it directly onto the copied-out psum blocks in
    SBUF, so DVE/ACT only do the psum->sbuf copies.  Three DMA paths run
    in parallel: sync ring (inputs), gpsimd ring (residual accumulate),
    scalar ring (output stores).
  * dummy matmuls keep the PE HAM clock warm while the initial DMAs land.

All 8 cores run one SPMD program whose loop structure (subtile layout) is
the per-row max across cores; it is rebuilt (and the NEFF recompiled) when
that structure changes, and cached for repeated calls with the same
structure.
"""

import numpy as np
import ml_dtypes

import concourse.bass as bass
import concourse.mybir as mybir
import concourse.tile as tile
from concourse.bass_utils import run_bass_kernel_spmd

# ---------------------------------------------------------------- constants
XMIN, XMAX, YMIN, YMAX = -80.0, 120.0, -40.0, 40.0
EPS = 1e-6
B, N, C, H, W = 4, 10000, 256, 200, 400
NCORES = 8
NPC = B * N // NCORES          # anchors per core
RK_MAX = 4096                  # bbox cap; beyond this fall back to host
SUBTILE = 512                  # max psum free width
OUTBLK = 2048                  # output block width (cols per store DMA)
WARMUP_MM = 10                 # dummy matmuls to keep the PE HAM-warm
BRIDGE_MM = 4                  # dummy matmuls between phase 1 and 2
F32 = mybir.dt.float32
BF16 = mybir.dt.bfloat16
NPBF16 = ml_dtypes.bfloat16

TRACE = False                  # set by test harness for profiling runs
LAST_RESULT = None             # BassKernelResults of the last device run

# --------------------------------------------------- walrus 1-wait workaround
# This container's walrus rejects >1 sem wait per instruction ("Too many
# sync wait commands").  Spread extra waits onto same-engine NoOps.

_MAXW = 1
_ctr = [0]


def _patched_drain_and_barrier(self, tick_clock, wait_clock):
    nc = self.nc
    probe = nc.sync.nop(hint="drain_wait_spread", nofuse=True)
    wait_clock.add_sem_waits(
        probe.ins, tile.ScopedClock({None: tick_clock.global_clock})
    )
    waits = list(probe.ins.sync_info.on_wait or [])
    if len(waits) > _MAXW:
        probe.ins.sync_info.on_wait = waits[:_MAXW]
        rest = waits[_MAXW:]
        while rest:
            chunk, rest = rest[:_MAXW], rest[_MAXW:]
            nxt = nc.sync.nop(hint="drain_wait_spread", nofuse=True)
            if nxt.ins.sync_info is None:
                nxt.ins.sync_info = mybir.SyncInfo(on_wait=chunk, on_update=[])
            else:
                nxt.ins.sync_info.on_wait = chunk
    nc.sync.drain()
    # One barrier (not two) before the semaphore cleanup; nothing runs after
    # the cleanup, so the trailing barrier of the stock tail is dropped.
    nc.all_engine_barrier()
    assert self.sems is not None
    popped = nc._tile_sem_poison_stack.pop()
    assert popped is self._sem_poison
    nc.clear_and_free_semaphores(list(self.sems.allocated().values()))


tile.TileContext._drain_and_barrier = _patched_drain_and_barrier


def _split_multiwait(nc):
    for f in nc.m.functions:
        for b in f.blocks:
            insts = list(b.instructions)
            out = []
            changed = False
            for inst in insts:
                si = inst.sync_info
                waits = list(si.on_wait) if (si and si.on_wait) else []
                if len(waits) > _MAXW:
                    changed = True
                    extra, keep = waits[:-_MAXW], waits[-_MAXW:]
                    si.on_wait = keep
                    inst.sync_info = si
                    for w in extra:
                        _ctr[0] += 1
                        nop = mybir.InstNoOp(
                            name=f"wsplit_{_ctr[0]}", ins=[], outs=[]
                        )
                        nop.engine = inst.engine
                        nop.sync_info = mybir.SyncInfo(on_wait=[w], on_update=[])
                        out.append(nop)
                out.append(inst)
            if changed:
                cur = b.instructions
                while len(cur):
                    cur.pop()
                for inst in out:
                    b.add_instruction(inst)


# ------------------------------------------------------------ device program
# structure = (rkp, Kw, ws, kch, n_pairs, nslot, subtiles); subtiles is a
# tuple of (pair_idx, col_offset, width).
_programs = {}


def _build_program(structure):
    rkp, Kw, ws, kch, n_pairs, nslot, subtiles = structure
    nc = bass.Bass()
    bevd = nc.declare_dram_parameter("bev", [C, rkp], BF16, isOutput=False)
    wptd = nc.declare_dram_parameter("wpt", [C, C], BF16, isOutput=False)
    wbd = nc.declare_dram_parameter("wb", [kch * 128, nslot], BF16,
                                    isOutput=False)
    ins = nc.declare_dram_parameter("instb_t", [C, nslot], BF16,
                                    isOutput=False)
    out = nc.declare_dram_parameter("out_t", [C, nslot], BF16, isOutput=True)

    with tile.TileContext(nc) as tc:
        with (
            tc.tile_pool(name="const", bufs=1) as constp,
            tc.tile_pool(name="ob", bufs=1) as obp,
            tc.tile_pool(name="ps", bufs=8, space="PSUM") as psp,
        ):
            # ---- PE warmup first: dummy matmuls on an uninitialized tile
            # (result never read) keep the HAM clock hot while DMAs land.
            wu = constp.tile([128, 512], BF16, tag="warm", name="warm")
            nc.gpsimd.memset(wu[:], 0.0)
            wups = psp.tile([128, SUBTILE], F32, tag="ps", name="wups")
            for _ in range(WARMUP_MM):
                nc.tensor.matmul(wups[:], lhsT=wu[:, 0:128], rhs=wu[:],
                                 start=True, stop=True)

            # ---- input DMAs on the sync (SP) HWDGE ring, which is FIFO:
            # bev+wpt gate phase 1, the wb column pieces gate phase 2's
            # first blocks, so order them that way.  The residual is NOT
            # loaded here: it arrives via accumulating SWDGE DMAs below.
            bev_sb = {}
            for cc in range(2):
                t = constp.tile([128, rkp], BF16, tag=f"bev{cc}",
                                name=f"bev{cc}")
                nc.sync.dma_start(t[:], bevd[cc * 128:(cc + 1) * 128, :])
                bev_sb[cc] = t
            wpt_sb = {}
            for cc in range(2):
                t = constp.tile([128, C], BF16, tag=f"wpt{cc}",
                                name=f"wpt{cc}")
                nc.sync.dma_start(t[:], wptd[cc * 128:(cc + 1) * 128, :])
                wpt_sb[cc] = t
            wsplit = min(OUTBLK, nslot)
            wb_sb = [constp.tile([128, nslot], BF16, tag=f"wb{ch}",
                                 name=f"wb{ch}") for ch in range(kch)]
            for s0, s1 in ((0, wsplit), (wsplit, nslot)):
                if s0 >= s1:
                    continue
                for ch in range(kch):
                    nc.sync.dma_start(
                        wb_sb[ch][:, s0:s1],
                        wbd[ch * 128:(ch + 1) * 128, s0:s1])

            # ---- phase 1: project row-pair windows into S' (bf16)
            # pair r covers subregion pixels [r*Kw, r*Kw + ws)
            sp = []
            for r in range(n_pairs):
                chs = []
                for ch in range(kch):
                    p0 = r * Kw + ch * 128
                    pw = max(0, min(128, ws - ch * 128, rkp - p0))
                    if pw == 0:
                        chs.append((None, 0))
                        continue
                    ps = psp.tile([128, SUBTILE], F32, tag="ps",
                                  name=f"ps1_{r}_{ch}")
                    for cc in range(2):
                        nc.tensor.matmul(
                            ps[0:pw, 0:C],
                            lhsT=bev_sb[cc][:, p0:p0 + pw],
                            rhs=wpt_sb[cc][:],
                            start=(cc == 0),
                            stop=(cc == 1),
                        )
                    t = constp.tile([128, C], BF16, tag=f"sp{r}_{ch}",
                                    name=f"sp{r}_{ch}")
                    if r % 2:
                        nc.vector.tensor_copy(t[0:pw, 0:C], ps[0:pw, 0:C])
                    else:
                        nc.scalar.copy(t[0:pw, 0:C], ps[0:pw, 0:C])
                    chs.append((t, pw))
                sp.append(chs)

            # bridge dummies: keep the PE busy while the wb DMAs land
            for _ in range(BRIDGE_MM):
                nc.tensor.matmul(wups[:], lhsT=wu[:, 0:128], rhs=wu[:],
                                 start=True, stop=True)

            # ---- phase 2: sampling matmuls.  Per (block, oc): psum is
            # copied out to the block's SBUF tile as soon as each subtile
            # finishes (freeing the bank), then one SWDGE accumulating DMA
            # adds the residual onto the block, then the scalar HWDGE ring
            # stores it.  Copies alternate DVE/ACT for throughput.
            blocks = []            # (b0, bw, [subtiles])
            for (r, c0, tw) in subtiles:
                if blocks and (c0 + tw - blocks[-1][0]) <= OUTBLK:
                    blocks[-1][2].append((r, c0, tw))
                    blocks[-1][1] = c0 + tw - blocks[-1][0]
                else:
                    blocks.append([c0, tw, [(r, c0, tw)]])

            sti = 0
            for b0, bw, sts in blocks:
                for oc in range(2):
                    ob = obp.tile([128, OUTBLK], BF16, tag=f"ob_{oc}_{b0}",
                                  name=f"ob_{oc}_{b0}")
                    for (r, c0, tw) in sts:
                        sti += 1
                        ps = psp.tile([128, SUBTILE], F32, tag="ps",
                                      name=f"ps2_{oc}_{c0}")
                        mms = [(sp[r][ch][0], sp[r][ch][1], ch)
                               for ch in range(kch) if sp[r][ch][1]]
                        for i, (t, pw, ch) in enumerate(mms):
                            nc.tensor.matmul(
                                ps[:, 0:tw],
                                lhsT=t[0:pw, oc * 128:(oc + 1) * 128],
                                rhs=wb_sb[ch][0:pw, c0:c0 + tw],
                                start=(i == 0),
                                stop=(i == len(mms) - 1),
                            )
                        lc = c0 - b0
                        if sti % 2:
                            nc.vector.tensor_copy(ob[:, lc:lc + tw],
                                                  ps[:, 0:tw])
                        else:
                            nc.scalar.copy(ob[:, lc:lc + tw], ps[:, 0:tw])
                    # residual: SWDGE accumulating load adds instb onto ob
                    nc.gpsimd.dma_start(
                        ob[:, 0:bw],
                        ins[oc * 128:(oc + 1) * 128, b0:b0 + bw],
                        accum_op=mybir.AluOpType.add,
                    )
                    nc.scalar.dma_start(
                        out[oc * 128:(oc + 1) * 128, b0:b0 + bw],
                        ob[:, 0:bw],
                    )

    return nc


def _get_program(structure):
    if structure not in _programs:
        nc = _build_program(structure)
        _split_multiwait(nc)
        nc._wsplit_done = True
        _programs[structure] = nc
    return _programs[structure]


# -------------------------------------------------------------- host prep
def _corners(anchor_bn):
    f = np.float32
    ax = anchor_bn[:, 0].astype(f)
    ay = anchor_bn[:, 1].astype(f)
    gx = (ax - f(XMIN)) / f(XMAX - XMIN + EPS) * f(2.0) - f(1.0)
    gy = (ay - f(YMIN)) / f(YMAX - YMIN + EPS) * f(2.0) - f(1.0)
    # module stacks [grid_y, grid_x]: width coord <- gy, height coord <- gx
    ix = (gy + f(1.0)) * f(0.5) * f(W - 1)
    iy = (gx + f(1.0)) * f(0.5) * f(H - 1)
    x0 = np.floor(ix)
    y0 = np.floor(iy)
    x1 = x0 + f(1.0)
    y1 = y0 + f(1.0)
    wx1 = ix - x0
    wx0 = f(1.0) - wx1
    wy1 = iy - y0
    wy0 = f(1.0) - wy1
    out = []
    for xc, yc, w in ((x0, y0, wx0 * wy0), (x1, y0, wx1 * wy0),
                      (x0, y1, wx0 * wy1), (x1, y1, wx1 * wy1)):
        valid = (xc >= 0) & (xc <= W - 1) & (yc >= 0) & (yc <= H - 1)
        xi = np.clip(xc, 0, W - 1).astype(np.int64)
        yi = np.clip(yc, 0, H - 1).astype(np.int64)
        out.append((xi, yi, valid, (w * valid.astype(f)).astype(f)))
    return out, y0


def _host_fallback(instance_feature, anchor, bev_map, W_proj, b_proj):
    """Exact numpy computation; only for pathological inputs whose bbox
    exceeds RK_MAX."""
    f = np.float32
    out = np.empty((B, N, C), f)
    for b in range(B):
        corners, _ = _corners(anchor[b])
        acc = np.zeros((N, C), f)
        fm = bev_map[b].reshape(C, H * W)
        for xi, yi, valid, w in corners:
            g = fm[:, yi * W + xi].T
            acc += g * w[:, None]
        out[b] = acc @ W_proj.T.astype(f) + b_proj.astype(f)
    return out + instance_feature.astype(f)


# ------------------------------------------------------------------- kernel
def kernel(instance_feature, anchor, anchor_embed, bev_map, W_proj, b_proj):
    global LAST_RESULT
    f = np.float32
    instance_feature = np.asarray(instance_feature)
    anchor = np.asarray(anchor)
    bev_map = np.asarray(bev_map)
    W_proj = np.asarray(W_proj)
    b_proj = np.asarray(b_proj)

    instb = instance_feature.astype(f) + b_proj.astype(f)[None, None, :]

    # ---- pass 1: per-core corner geometry
    cores = []
    for core in range(NCORES):
        b, half = core // 2, core % 2
        sl = slice(half * NPC, (half + 1) * NPC)
        corners, y0f = _corners(anchor[b, sl])
        vx = np.concatenate([np.where(v, xi, -1) for xi, yi, v, w in corners])
        vy = np.concatenate([np.where(v, yi, -1) for xi, yi, v, w in corners])
        m = vx >= 0
        if m.any():
            xmin, xmax = int(vx[m].min()), int(vx[m].max())
            ymin, ymax = int(vy[m].min()), int(vy[m].max())
        else:
            xmin = xmax = ymin = ymax = 0
        R, K = ymax - ymin + 1, xmax - xmin + 1
        if R * K > RK_MAX:
            return _host_fallback(instance_feature, anchor, bev_map,
                                  W_proj, b_proj)
        cores.append((corners, y0f, xmin, ymin, R, K))

    # ---- unified structure
    Kw = max(c[5] for c in cores)
    n_pairs = max(max(c[4] - 1, 1) for c in cores)
    ws = 2 * Kw
    kch = -(-ws // 128)
    rkp = 128 * -(-max((n_pairs - 1) * Kw + ws,
                       max(c[4] * Kw for c in cores)) // 128)
    if rkp > RK_MAX:
        return _host_fallback(instance_feature, anchor, bev_map,
                              W_proj, b_proj)

    y0ps = []
    counts = np.zeros((NCORES, n_pairs), np.int64)
    for core, (corners, y0f, xmin, ymin, R, K) in enumerate(cores):
        y0p = np.clip(y0f.astype(np.int64) - ymin, 0, max(R - 2, 0))
        y0p = np.minimum(y0p, n_pairs - 1)
        y0ps.append(y0p)
        counts[core] = np.bincount(y0p, minlength=n_pairs)
    cap = counts.max(axis=0)

    subtiles = []
    c0 = 0
    for r in range(n_pairs):
        left = int(cap[r])
        while left > 0:
            tw = min(SUBTILE, left)
            subtiles.append((r, c0, tw))
            c0 += tw
            left -= tw
    nslot = c0
    structure = (rkp, Kw, ws, kch, n_pairs, nslot, tuple(subtiles))

    # ---- pass 2: per-core arrays against the unified layout
    row_base = {}
    base = 0
    for r in range(n_pairs):
        row_base[r] = base
        base += int(cap[r])

    maps, perms = [], []
    wpt = np.ascontiguousarray(W_proj.astype(f).T).astype(NPBF16)
    for core, (corners, y0f, xmin, ymin, R, K) in enumerate(cores):
        b, half = core // 2, core % 2
        sl = slice(half * NPC, (half + 1) * NPC)
        y0p = y0ps[core]
        # stable sort by pair row; columns are packed at each row's base
        order = np.argsort(y0p, kind="stable")
        cnt = counts[core]
        col_of = np.empty(NPC, np.int64)
        start = 0
        for r in range(n_pairs):
            end = start + int(cnt[r])
            col_of[order[start:end]] = row_base[r] + np.arange(end - start)
            start = end

        bev_sub = np.zeros((C, rkp), f)
        ke = min(xmin + Kw, W)
        bev_rows = bev_map[b][:, ymin:ymin + R, xmin:ke].astype(f)
        tmp = np.zeros((C, R, Kw), f)
        tmp[:, :, :ke - xmin] = bev_rows
        bev_sub[:, :R * Kw] = tmp.reshape(C, R * Kw)

        wb = np.zeros((kch * 128, nslot), NPBF16)
        for xi, yi, valid, wgt in corners:
            px = (yi - ymin - y0p) * Kw + (xi - xmin)
            wb[px[valid], col_of[valid]] = wgt[valid].astype(NPBF16)

        instb_t = np.zeros((C, nslot), NPBF16)
        instb_t[:, col_of] = instb[b, sl].T.astype(NPBF16)

        maps.append({
            "bev": bev_sub.astype(NPBF16),
            "wb": wb,
            "instb_t": instb_t,
            "wpt": wpt,
        })
        perms.append(col_of)

    nc = _get_program(structure)
    res = run_bass_kernel_spmd(nc, maps, list(range(NCORES)), trace=TRACE)
    LAST_RESULT = res

    out = np.empty((B, N, C), f)
    for core in range(NCORES):
        b, half = core // 2, core % 2
        sl = slice(half * NPC, (half + 1) * NPC)
        o = res.results[core]["out_t"]
        out[b, sl] = o[:, perms[core]].T.astype(f)
    return out


# revision 3
# speedup vs baseline: 1.8385x; 1.5636x over previous
"""BEVFeatureAggregation Trainium2 kernel.

Math: out[b,n,o] = inst[b,n,o] + b_proj[o]
                 + sum_c W_proj[o,c] * bilinear_sample(bev_map[b], anchor[b,n])[c]

Strategy (8 NeuronCores, core = batch*2 + anchor-half, 5000 anchors each):
  * anchors concentrate in a tiny window of the 200x400 BEV map; the host
    computes the bounding box (R rows x K cols) of all touched bilinear
    corners and ships only that subregion (C x R*K, zero-padded) per core.
    The row origin is GLOBAL (min over cores) so the per-row anchor
    distributions align across cores and the shared column layout has
    minimal padding.
  * the device projects the subregion first:  S'[px,o] = sum_c sub[c,px] *
    W_proj[o,c], so sampling directly produces output features.
  * the host sorts anchors into row GROUPS of rpw=128//Kw consecutive BEV
    rows (un-permuting on the way out).  All 4 corners of an anchor in
    group g live in the rpw*Kw <= 128 pixel window starting at row
    g*(rpw-1), so each group's sampling is a dense matmul with contraction
    over that window only:
        out_T[o, n] = sum_px S'g[px, o] * wb[px, n]
    wb (<=128 x NSLOT) holds the 4 bilinear corner weights per column.
  * everything is single bf16 (tolerance is 2e-2; measured pipeline error
    ~5e-3), which halves HBM traffic vs fp32 and needs 1 matmul pass.
  * the residual (instance_feature + b_proj, transposed and permuted on
    host, bf16) is added BY THE PE: each psum subtile is seeded with an
    identity matmul  ps = I.T @ instb  (start=True) and the sampling
    matmuls accumulate on top, so the engines only do plain psum->sbuf
    copies (alternating DVE/ACT) and no separate add pass exists.
  * dummy matmuls keep the PE HAM clock warm while the initial DMAs land.

All 8 cores run one SPMD program whose loop structure (subtile layout) is
the per-group max across cores; it is rebuilt (and the NEFF recompiled)
when that structure changes, and cached for repeated calls with the same
structure.
"""

import numpy as np
import ml_dtypes

import concourse.bass as bass
import concourse.mybir as mybir
import concourse.tile as tile
from concourse.bass_utils import run_bass_kernel_spmd

# ---------------------------------------------------------------- constants
XMIN, XMAX, YMIN, YMAX = -80.0, 120.0, -40.0, 40.0
EPS = 1e-6
B, N, C, H, W = 4, 10000, 256, 200, 400
NCORES = 8
NPC = B * N // NCORES          # anchors per core
RK_MAX = 4096                  # bbox cap; beyond this fall back to host
SUBTILE = 512                  # max psum free width
OUTBLK = 2048                  # output block width (cols per store DMA)
WARMUP_MM = 10                 # dummy matmuls to keep the PE HAM-warm
BRIDGE_MM = 4                  # dummy matmuls between phase 1 and 2
F32 = mybir.dt.float32
BF16 = mybir.dt.bfloat16
NPBF16 = ml_dtypes.bfloat16

TRACE = False                  # set by test harness for profiling runs
LAST_RESULT = None             # BassKernelResults of the last device run

# --------------------------------------------------- walrus 1-wait workaround
# This container's walrus rejects >1 sem wait per instruction ("Too many
# sync wait commands").  Spread extra waits onto same-engine NoOps.

_MAXW = 1
_ctr = [0]


def _patched_drain_and_barrier(self, tick_clock, wait_clock):
    nc = self.nc
    probe = nc.sync.nop(hint="drain_wait_spread", nofuse=True)
    wait_clock.add_sem_waits(
        probe.ins, tile.ScopedClock({None: tick_clock.global_clock})
    )
    waits = list(probe.ins.sync_info.on_wait or [])
    if len(waits) > _MAXW:
        probe.ins.sync_info.on_wait = waits[:_MAXW]
        rest = waits[_MAXW:]
        while rest:
            chunk, rest = rest[:_MAXW], rest[_MAXW:]
            nxt = nc.sync.nop(hint="drain_wait_spread", nofuse=True)
            if nxt.ins.sync_info is None:
                nxt.ins.sync_info = mybir.SyncInfo(on_wait=chunk, on_update=[])
            else:
                nxt.ins.sync_info.on_wait = chunk
    nc.sync.drain()
    # One barrier (not two) before the semaphore cleanup; nothing runs after
    # the cleanup, so the trailing barrier of the stock tail is dropped.
    nc.all_engine_barrier()
    assert self.sems is not None
    popped = nc._tile_sem_poison_stack.pop()
    assert popped is self._sem_poison
    nc.clear_and_free_semaphores(list(self.sems.allocated().values()))


tile.TileContext._drain_and_barrier = _patched_drain_and_barrier


def _split_multiwait(nc):
    for f in nc.m.functions:
        for b in f.blocks:
            insts = list(b.instructions)
            out = []
            changed = False
            for inst in insts:
                si = inst.sync_info
                waits = list(si.on_wait) if (si and si.on_wait) else []
                if len(waits) > _MAXW:
                    changed = True
                    extra, keep = waits[:-_MAXW], waits[-_MAXW:]
                    si.on_wait = keep
                    inst.sync_info = si
                    for w in extra:
                        _ctr[0] += 1
                        nop = mybir.InstNoOp(
                            name=f"wsplit_{_ctr[0]}", ins=[], outs=[]
                        )
                        nop.engine = inst.engine
                        nop.sync_info = mybir.SyncInfo(on_wait=[w], on_update=[])
                        out.append(nop)
                out.append(inst)
            if changed:
                cur = b.instructions
                while len(cur):
                    cur.pop()
                for inst in out:
                    b.add_instruction(inst)


# ------------------------------------------------------------ device program
# structure = (rkp, Kw, ws, stride, kch, n_groups, nslot, subtiles);
# subtiles is a tuple of (group_idx, col_offset, width).
_programs = {}


def _build_program(structure):
    rkp, Kw, ws, stride, kch, n_groups, nslot, subtiles = structure
    nc = bass.Bass()
    bevd = nc.declare_dram_parameter("bev", [C, rkp], BF16, isOutput=False)
    wptd = nc.declare_dram_parameter("wpt", [C, C], BF16, isOutput=False)
    identd = nc.declare_dram_parameter("ident", [128, 128], BF16,
                                       isOutput=False)
    wbd = nc.declare_dram_parameter("wb", [kch * 128, nslot], BF16,
                                    isOutput=False)
    ins = nc.declare_dram_parameter("instb_t", [C, nslot], BF16,
                                    isOutput=False)
    out = nc.declare_dram_parameter("out_t", [C, nslot], BF16, isOutput=True)

    with tile.TileContext(nc) as tc:
        with (
            tc.tile_pool(name="const", bufs=1) as constp,
            tc.tile_pool(name="ob", bufs=1) as obp,
            tc.tile_pool(name="ps", bufs=8, space="PSUM") as psp,
        ):
            # ---- PE warmup first: dummy matmuls on an uninitialized tile
            # (result never read) keep the HAM clock hot while DMAs land.
            wu = constp.tile([128, 512], BF16, tag="warm", name="warm")
            nc.gpsimd.memset(wu[:], 0.0)
            wups = psp.tile([128, SUBTILE], F32, tag="ps", name="wups")
            for _ in range(WARMUP_MM):
                nc.tensor.matmul(wups[:], lhsT=wu[:, 0:128], rhs=wu[:],
                                 start=True, stop=True)

            # ---- input DMAs on the sync (SP) HWDGE ring, which is FIFO:
            # bev+wpt gate phase 1; wb/instb column pieces gate phase 2
            # block by block, so they are interleaved in block order.
            bev_sb = {}
            for cc in range(2):
                t = constp.tile([128, rkp], BF16, tag=f"bev{cc}",
                                name=f"bev{cc}")
                nc.sync.dma_start(t[:], bevd[cc * 128:(cc + 1) * 128, :])
                bev_sb[cc] = t
            wpt_sb = {}
            for cc in range(2):
                t = constp.tile([128, C], BF16, tag=f"wpt{cc}",
                                name=f"wpt{cc}")
                nc.sync.dma_start(t[:], wptd[cc * 128:(cc + 1) * 128, :])
                wpt_sb[cc] = t
            ident = constp.tile([128, 128], BF16, tag="ident", name="ident")
            nc.sync.dma_start(ident[:], identd[:, :])

            wb_sb = [constp.tile([128, nslot], BF16, tag=f"wb{ch}",
                                 name=f"wb{ch}") for ch in range(kch)]
            inst_sb = [constp.tile([128, nslot], BF16, tag=f"instb{oc}",
                                   name=f"instb{oc}") for oc in range(2)]
            pieces = []
            s0 = 0
            while s0 < nslot:
                s1 = min(s0 + OUTBLK, nslot)
                pieces.append((s0, s1))
                s0 = s1
            for s0, s1 in pieces:
                for ch in range(kch):
                    nc.sync.dma_start(
                        wb_sb[ch][:, s0:s1],
                        wbd[ch * 128:(ch + 1) * 128, s0:s1])
                for oc in range(2):
                    nc.sync.dma_start(
                        inst_sb[oc][:, s0:s1],
                        ins[oc * 128:(oc + 1) * 128, s0:s1])

            # ---- phase 1: project row-group windows into S' (bf16)
            # group g covers subregion pixels [g*stride*Kw, g*stride*Kw+ws)
            sp = []
            for g in range(n_groups):
                chs = []
                for ch in range(kch):
                    p0 = g * stride * Kw + ch * 128
                    pw = max(0, min(128, ws - ch * 128, rkp - p0))
                    if pw == 0:
                        chs.append((None, 0))
                        continue
                    ps = psp.tile([128, SUBTILE], F32, tag="ps",
                                  name=f"ps1_{g}_{ch}")
                    for cc in range(2):
                        nc.tensor.matmul(
                            ps[0:pw, 0:C],
                            lhsT=bev_sb[cc][:, p0:p0 + pw],
                            rhs=wpt_sb[cc][:],
                            start=(cc == 0),
                            stop=(cc == 1),
                        )
                    t = constp.tile([128, C], BF16, tag=f"sp{g}_{ch}",
                                    name=f"sp{g}_{ch}")
                    if g % 2:
                        nc.vector.tensor_copy(t[0:pw, 0:C], ps[0:pw, 0:C])
                    else:
                        nc.scalar.copy(t[0:pw, 0:C], ps[0:pw, 0:C])
                    chs.append((t, pw))
                sp.append(chs)

            # bridge dummies: keep the PE busy while the wb DMAs land
            for _ in range(BRIDGE_MM):
                nc.tensor.matmul(wups[:], lhsT=wu[:, 0:128], rhs=wu[:],
                                 start=True, stop=True)

            # ---- phase 2: per subtile, seed psum with the residual via an
            # identity matmul, accumulate the sampling matmuls on top, then
            # one plain psum->sbuf copy (alternating DVE/ACT).  Blocks of
            # OUTBLK columns go out on the scalar HWDGE ring as they finish.
            blocks = []            # (b0, bw, [subtiles])
            for (g, c0, tw) in subtiles:
                if blocks and (c0 + tw - blocks[-1][0]) <= OUTBLK:
                    blocks[-1][2].append((g, c0, tw))
                    blocks[-1][1] = c0 + tw - blocks[-1][0]
                else:
                    blocks.append([c0, tw, [(g, c0, tw)]])

            sti = 0
            for b0, bw, sts in blocks:
                for oc in range(2):
                    ob = obp.tile([128, OUTBLK], BF16, tag=f"ob_{oc}_{b0}",
                                  name=f"ob_{oc}_{b0}")
                    for (g, c0, tw) in sts:
                        sti += 1
                        ps = psp.tile([128, SUBTILE], F32, tag="ps",
                                      name=f"ps2_{oc}_{c0}")
                        nc.tensor.matmul(
                            ps[:, 0:tw],
                            lhsT=ident[:],
                            rhs=inst_sb[oc][:, c0:c0 + tw],
                            start=True, stop=False,
                        )
                        mms = [(sp[g][ch][0], sp[g][ch][1], ch)
                               for ch in range(kch) if sp[g][ch][1]]
                        for i, (t, pw, ch) in enumerate(mms):
                            nc.tensor.matmul(
                                ps[:, 0:tw],
                                lhsT=t[0:pw, oc * 128:(oc + 1) * 128],
                                rhs=wb_sb[ch][0:pw, c0:c0 + tw],
                                start=False,
                                stop=(i == len(mms) - 1),
                            )
                        lc = c0 - b0
                        if sti % 2:
                            nc.vector.tensor_copy(ob[:, lc:lc + tw],
                                                  ps[:, 0:tw])
                        else:
                            nc.scalar.copy(ob[:, lc:lc + tw], ps[:, 0:tw])
                    nc.scalar.dma_start(
                        out[oc * 128:(oc + 1) * 128, b0:b0 + bw],
                        ob[:, 0:bw],
                    )

    return nc


def _get_program(structure):
    if structure not in _programs:
        nc = _build_program(structure)
        _split_multiwait(nc)
        nc._wsplit_done = True
        _programs[structure] = nc
    return _programs[structure]


# -------------------------------------------------------------- host prep
def _corners(anchor_bn):
    f = np.float32
    ax = anchor_bn[:, 0].astype(f)
    ay = anchor_bn[:, 1].astype(f)
    gx = (ax - f(XMIN)) / f(XMAX - XMIN + EPS) * f(2.0) - f(1.0)
    gy = (ay - f(YMIN)) / f(YMAX - YMIN + EPS) * f(2.0) - f(1.0)
    # module stacks [grid_y, grid_x]: width coord <- gy, height coord <- gx
    ix = (gy + f(1.0)) * f(0.5) * f(W - 1)
    iy = (gx + f(1.0)) * f(0.5) * f(H - 1)
    x0 = np.floor(ix)
    y0 = np.floor(iy)
    x1 = x0 + f(1.0)
    y1 = y0 + f(1.0)
    wx1 = ix - x0
    wx0 = f(1.0) - wx1
    wy1 = iy - y0
    wy0 = f(1.0) - wy1
    out = []
    for xc, yc, w in ((x0, y0, wx0 * wy0), (x1, y0, wx1 * wy0),
                      (x0, y1, wx0 * wy1), (x1, y1, wx1 * wy1)):
        valid = (xc >= 0) & (xc <= W - 1) & (yc >= 0) & (yc <= H - 1)
        xi = np.clip(xc, 0, W - 1).astype(np.int64)
        yi = np.clip(yc, 0, H - 1).astype(np.int64)
        out.append((xi, yi, valid, (w * valid.astype(f)).astype(f)))
    return out, y0


def _host_fallback(instance_feature, anchor, bev_map, W_proj, b_proj):
    """Exact numpy computation; only for pathological inputs whose bbox
    exceeds RK_MAX."""
    f = np.float32
    out = np.empty((B, N, C), f)
    for b in range(B):
        corners, _ = _corners(anchor[b])
        acc = np.zeros((N, C), f)
        fm = bev_map[b].reshape(C, H * W)
        for xi, yi, valid, w in corners:
            g = fm[:, yi * W + xi].T
            acc += g * w[:, None]
        out[b] = acc @ W_proj.T.astype(f) + b_proj.astype(f)
    return out + instance_feature.astype(f)


# ------------------------------------------------------------------- kernel
def kernel(instance_feature, anchor, anchor_embed, bev_map, W_proj, b_proj):
    global LAST_RESULT
    f = np.float32
    instance_feature = np.asarray(instance_feature)
    anchor = np.asarray(anchor)
    bev_map = np.asarray(bev_map)
    W_proj = np.asarray(W_proj)
    b_proj = np.asarray(b_proj)

    instb = instance_feature.astype(f) + b_proj.astype(f)[None, None, :]

    # ---- pass 1: per-core corner geometry
    cores = []
    for core in range(NCORES):
        b, half = core // 2, core % 2
        sl = slice(half * NPC, (half + 1) * NPC)
        corners, y0f = _corners(anchor[b, sl])
        vx = np.concatenate([np.where(v, xi, -1) for xi, yi, v, w in corners])
        vy = np.concatenate([np.where(v, yi, -1) for xi, yi, v, w in corners])
        m = vx >= 0
        if m.any():
            xmin, xmax = int(vx[m].min()), int(vx[m].max())
            ymin, ymax = int(vy[m].min()), int(vy[m].max())
        else:
            xmin = xmax = ymin = ymax = 0
        if (ymax - ymin + 1) * (xmax - xmin + 1) > RK_MAX:
            return _host_fallback(instance_feature, anchor, bev_map,
                                  W_proj, b_proj)
        cores.append((corners, y0f, xmin, xmax, ymin, ymax))

    # ---- unified structure: GLOBAL row origin so core layouts align
    ymin_g = min(c[4] for c in cores)
    ymax_g = max(c[5] for c in cores)
    Rg = ymax_g - ymin_g + 1
    Kw = max(c[3] - c[2] + 1 for c in cores)
    rpw = max(2, min(128 // max(Kw, 1), Rg)) if Kw <= 64 else 2
    stride = rpw - 1
    n_groups = max(Rg - 2, 0) // stride + 1
    ws = rpw * Kw
    kch = -(-ws // 128)
    rkp = 128 * -(-max(Rg * Kw, (n_groups - 1) * stride * Kw + ws) // 128)
    if rkp > RK_MAX:
        return _host_fallback(instance_feature, anchor, bev_map,
                              W_proj, b_proj)

    y0ps, gs = [], []
    counts = np.zeros((NCORES, n_groups), np.int64)
    for core, (corners, y0f, xmin, xmax, ymin, ymax) in enumerate(cores):
        y0p = np.clip(y0f.astype(np.int64) - ymin_g, 0, max(Rg - 2, 0))
        grp = np.minimum(y0p // stride, n_groups - 1)
        y0ps.append(y0p)
        gs.append(grp)
        counts[core] = np.bincount(grp, minlength=n_groups)
    cap = counts.max(axis=0)

    subtiles = []
    c0 = 0
    for g in range(n_groups):
        left = int(cap[g])
        while left > 0:
            tw = min(SUBTILE, left)
            subtiles.append((g, c0, tw))
            c0 += tw
            left -= tw
    nslot = c0
    structure = (rkp, Kw, ws, stride, kch, n_groups, nslot, tuple(subtiles))

    # ---- pass 2: per-core arrays against the unified layout
    row_base = {}
    base = 0
    for g in range(n_groups):
        row_base[g] = base
        base += int(cap[g])

    maps, perms = [], []
    wpt = np.ascontiguousarray(W_proj.astype(f).T).astype(NPBF16)
    identity = np.eye(128, dtype=NPBF16)
    for core, (corners, y0f, xmin, xmax, ymin, ymax) in enumerate(cores):
        b, half = core // 2, core % 2
        sl = slice(half * NPC, (half + 1) * NPC)
        grp = gs[core]
        # stable sort by group; columns are packed at each group's base
        order = np.argsort(grp, kind="stable")
        cnt = counts[core]
        col_of = np.empty(NPC, np.int64)
        start = 0
        for g in range(n_groups):
            end = start + int(cnt[g])
            col_of[order[start:end]] = row_base[g] + np.arange(end - start)
            start = end

        bev_sub = np.zeros((C, rkp), f)
        ke = min(xmin + Kw, W)
        ye = min(ymin_g + Rg, H)
        bev_rows = bev_map[b][:, ymin_g:ye, xmin:ke].astype(f)
        tmp = np.zeros((C, Rg, Kw), f)
        tmp[:, :ye - ymin_g, :ke - xmin] = bev_rows
        bev_sub[:, :Rg * Kw] = tmp.reshape(C, Rg * Kw)

        wb = np.zeros((kch * 128, nslot), NPBF16)
        for xi, yi, valid, wgt in corners:
            px = (yi - ymin_g - grp * stride) * Kw + (xi - xmin)
            wb[px[valid], col_of[valid]] = wgt[valid].astype(NPBF16)

        instb_t = np.zeros((C, nslot), NPBF16)
        instb_t[:, col_of] = instb[b, sl].T.astype(NPBF16)

        maps.append({
            "bev": bev_sub.astype(NPBF16),
            "wb": wb,
            "instb_t": instb_t,
            "wpt": wpt,
            "ident": identity,
        })
        perms.append(col_of)

    nc = _get_program(structure)
    res = run_bass_kernel_spmd(nc, maps, list(range(NCORES)), trace=TRACE)
    LAST_RESULT = res

    out = np.empty((B, N, C), f)
    for core in range(NCORES):
        b, half = core // 2, core % 2
        sl = slice(half * NPC, (half + 1) * NPC)
        o = res.results[core]["out_t"]
        out[b, sl] = o[:, perms[core]].T.astype(f)
    return out


# revision 12
# speedup vs baseline: 2.0309x; 1.1046x over previous
"""BEVFeatureAggregation Trainium2 kernel.

Math: out[b,n,o] = inst[b,n,o] + b_proj[o]
                 + sum_c W_proj[o,c] * bilinear_sample(bev_map[b], anchor[b,n])[c]

Strategy (8 NeuronCores, core = batch*2 + anchor-half, 5000 anchors each):
  * anchors concentrate in a tiny window of the 200x400 BEV map; the host
    computes the bounding box (R rows x K cols) of all touched bilinear
    corners and ships only that subregion (C x R*K, zero-padded) per core.
    The row origin is GLOBAL (min over cores) so the per-row anchor
    distributions align across cores and the shared column layout has
    minimal padding.
  * the device projects the subregion first:  S'[px,o] = sum_c sub[c,px] *
    W_proj[o,c], so sampling directly produces output features.
  * the host sorts anchors into row GROUPS of rpw=128//Kw consecutive BEV
    rows (un-permuting on the way out).  All 4 corners of an anchor in
    group g live in the rpw*Kw <= 128 pixel window starting at row
    g*(rpw-1), so each group's sampling is a dense matmul with contraction
    over that window only:
        out_T[o, n] = sum_px S'g[px, o] * wb[px, n]
    wb (<=128 x NSLOT) holds the 4 bilinear corner weights per column.
  * everything is single bf16 (tolerance is 2e-2; measured pipeline error
    ~5e-3), which halves HBM traffic vs fp32 and needs 1 matmul pass.
  * the residual (instance_feature + b_proj, transposed and permuted on
    host, bf16) is added BY THE PE: each psum subtile is seeded with an
    identity matmul  ps = I.T @ instb  (start=True) and the sampling
    matmuls accumulate on top, so the engines only do plain psum->sbuf
    copies (alternating DVE/ACT) and no separate add pass exists.
  * dummy matmuls keep the PE HAM clock warm while the initial DMAs land.

All 8 cores run one SPMD program whose loop structure (subtile layout) is
the per-group max across cores; it is rebuilt (and the NEFF recompiled)
when that structure changes, and cached for repeated calls with the same
structure.
"""

import numpy as np
import ml_dtypes

import concourse.bass as bass
import concourse.mybir as mybir
import concourse.tile as tile
from concourse.bass_utils import run_bass_kernel_spmd

# ---------------------------------------------------------------- constants
XMIN, XMAX, YMIN, YMAX = -80.0, 120.0, -40.0, 40.0
EPS = 1e-6
B, N, C, H, W = 4, 10000, 256, 200, 400
NCORES = 8
NPC = B * N // NCORES          # anchors per core
RK_MAX = 4096                  # bbox cap; beyond this fall back to host
SUBTILE = 512                  # max psum free width
OUTBLK = 2048                  # output block width (cols per store DMA)
WARMUP_MM = 10                 # dummy matmuls to keep the PE HAM-warm
BRIDGE_MM = 4                  # dummy matmuls between phase 1 and 2
F32 = mybir.dt.float32
BF16 = mybir.dt.bfloat16
NPBF16 = ml_dtypes.bfloat16

TRACE = False                  # set by test harness for profiling runs
LAST_RESULT = None             # BassKernelResults of the last device run

# --------------------------------------------------- walrus 1-wait workaround
# This container's walrus rejects >1 sem wait per instruction ("Too many
# sync wait commands").  Spread extra waits onto same-engine NoOps.

_MAXW = 1
_ctr = [0]


def _patched_drain_and_barrier(self, tick_clock, wait_clock):
    nc = self.nc
    probe = nc.sync.nop(hint="drain_wait_spread", nofuse=True)
    wait_clock.add_sem_waits(
        probe.ins, tile.ScopedClock({None: tick_clock.global_clock})
    )
    waits = list(probe.ins.sync_info.on_wait or [])
    if len(waits) > _MAXW:
        probe.ins.sync_info.on_wait = waits[:_MAXW]
        rest = waits[_MAXW:]
        while rest:
            chunk, rest = rest[:_MAXW], rest[_MAXW:]
            nxt = nc.sync.nop(hint="drain_wait_spread", nofuse=True)
            if nxt.ins.sync_info is None:
                nxt.ins.sync_info = mybir.SyncInfo(on_wait=chunk, on_update=[])
            else:
                nxt.ins.sync_info.on_wait = chunk
    nc.sync.drain()
    # One barrier (not two) before the semaphore cleanup; nothing runs after
    # the cleanup, so the trailing barrier of the stock tail is dropped.
    nc.all_engine_barrier()
    assert self.sems is not None
    popped = nc._tile_sem_poison_stack.pop()
    assert popped is self._sem_poison
    nc.clear_and_free_semaphores(list(self.sems.allocated().values()))


tile.TileContext._drain_and_barrier = _patched_drain_and_barrier


def _split_multiwait(nc):
    for f in nc.m.functions:
        for b in f.blocks:
            insts = list(b.instructions)
            out = []
            changed = False
            for inst in insts:
                si = inst.sync_info
                waits = list(si.on_wait) if (si and si.on_wait) else []
                if len(waits) > _MAXW:
                    changed = True
                    extra, keep = waits[:-_MAXW], waits[-_MAXW:]
                    si.on_wait = keep
                    inst.sync_info = si
                    for w in extra:
                        _ctr[0] += 1
                        nop = mybir.InstNoOp(
                            name=f"wsplit_{_ctr[0]}", ins=[], outs=[]
                        )
                        nop.engine = inst.engine
                        nop.sync_info = mybir.SyncInfo(on_wait=[w], on_update=[])
                        out.append(nop)
                out.append(inst)
            if changed:
                cur = b.instructions
                while len(cur):
                    cur.pop()
                for inst in out:
                    b.add_instruction(inst)


# ------------------------------------------------------------ device program
# structure = (rkp, Kw, ws, stride, kch, n_groups, nslot, subtiles);
# subtiles is a tuple of (group_idx, col_offset, width).
_programs = {}


def _build_program(structure):
    rkp, Kw, ws, stride, kch, n_groups, nslot, subtiles = structure
    cw = rkp + C + 128           # packed consts: bev | wptT | identity
    nc = bass.Bass()
    constd = nc.declare_dram_parameter("consts", [C, cw], BF16,
                                       isOutput=False)
    wbd = nc.declare_dram_parameter("wb", [kch * 128, nslot], BF16,
                                    isOutput=False)
    ins = nc.declare_dram_parameter("instb_t", [C, nslot], BF16,
                                    isOutput=False)
    out = nc.declare_dram_parameter("out_t", [C, nslot], BF16, isOutput=True)

    # output blocks (whole subtiles, <= OUTBLK cols each)
    blocks = []            # (b0, bw, [subtiles])
    for (g, c0, tw) in subtiles:
        if blocks and (c0 + tw - blocks[-1][0]) <= OUTBLK:
            blocks[-1][2].append((g, c0, tw))
            blocks[-1][1] = c0 + tw - blocks[-1][0]
        else:
            blocks.append([c0, tw, [(g, c0, tw)]])
    # input column pieces: ~thirds, aligned to block starts so no block
    # waits on two pieces
    bounds = sorted({b0 for b0, _, _ in blocks} | {nslot})
    splits = []
    for frac in (1 / 3, 2 / 3):
        tgt = int(nslot * frac)
        cand = min(bounds, key=lambda x: abs(x - tgt))
        if cand not in (0, nslot) and cand not in splits:
            splits.append(cand)
    pieces = []
    lo = 0
    for s in sorted(splits) + [nslot]:
        if s > lo:
            pieces.append((lo, s))
            lo = s

    with tile.TileContext(nc) as tc:
        with (
            tc.tile_pool(name="const", bufs=1) as constp,
            tc.tile_pool(name="ob", bufs=1) as obp,
            tc.tile_pool(name="ps", bufs=8, space="PSUM") as psp,
        ):
            # ---- PE warmup first: dummy matmuls on a DVE-memset tile keep
            # the HAM clock hot while DMAs land; a 1-col ACT copy pulls the
            # activation table load off the critical path.
            wu = constp.tile([128, 512], BF16, tag="warm", name="warm")
            nc.vector.memset(wu[:], 0.0)
            aw = constp.tile([128, 1], BF16, tag="actwarm", name="actwarm")
            nc.scalar.copy(aw[:], wu[:, 0:1])
            wups = psp.tile([128, SUBTILE], F32, tag="ps", name="wups")
            for _ in range(WARMUP_MM):
                nc.tensor.matmul(wups[:], lhsT=wu[:, 0:128], rhs=wu[:],
                                 start=True, stop=True)

            # ---- input DMAs.  Each dma_start costs ~650ns of sequencer
            # issue time, so they are batched (consts packed as one tensor)
            # and spread over both HWDGE rings: sync gets consts + wb +
            # instb oc0 (and later the stores), scalar gets instb oc1.
            # Pieces stream in block order so compute starts early.
            const_sb = {}
            for cc in range(2):
                t = constp.tile([128, cw], BF16, tag=f"const{cc}",
                                name=f"const{cc}")
                nc.sync.dma_start(t[:], constd[cc * 128:(cc + 1) * 128, :])
                const_sb[cc] = t

            wb_sb = [constp.tile([128, nslot], BF16, tag=f"wb{ch}",
                                 name=f"wb{ch}") for ch in range(kch)]
            inst_sb = [constp.tile([128, nslot], BF16, tag=f"instb{oc}",
                                   name=f"instb{oc}") for oc in range(2)]
            for s0, s1 in pieces:
                for ch in range(kch):
                    nc.sync.dma_start(
                        wb_sb[ch][:, s0:s1],
                        wbd[ch * 128:(ch + 1) * 128, s0:s1])
                nc.sync.dma_start(inst_sb[0][:, s0:s1], ins[0:128, s0:s1])
                nc.scalar.dma_start(inst_sb[1][:, s0:s1],
                                    ins[128:256, s0:s1])

            # ---- phase 1: project row-group windows into S' (bf16)
            # group g covers subregion pixels [g*stride*Kw, g*stride*Kw+ws)
            sp = []
            for g in range(n_groups):
                chs = []
                for ch in range(kch):
                    p0 = g * stride * Kw + ch * 128
                    pw = max(0, min(128, ws - ch * 128, rkp - p0))
                    if pw == 0:
                        chs.append((None, 0))
                        continue
                    ps = psp.tile([128, SUBTILE], F32, tag="ps",
                                  name=f"ps1_{g}_{ch}")
                    for cc in range(2):
                        nc.tensor.matmul(
                            ps[0:pw, 0:C],
                            lhsT=const_sb[cc][:, p0:p0 + pw],
                            rhs=const_sb[cc][:, rkp:rkp + C],
                            start=(cc == 0),
                            stop=(cc == 1),
                        )
                    t = constp.tile([128, C], BF16, tag=f"sp{g}_{ch}",
                                    name=f"sp{g}_{ch}")
                    if g % 2:
                        nc.vector.tensor_copy(t[0:pw, 0:C], ps[0:pw, 0:C])
                    else:
                        nc.scalar.copy(t[0:pw, 0:C], ps[0:pw, 0:C])
                    chs.append((t, pw))
                sp.append(chs)

            # bridge dummies: keep the PE busy while the wb DMAs land
            for _ in range(BRIDGE_MM):
                nc.tensor.matmul(wups[:], lhsT=wu[:, 0:128], rhs=wu[:],
                                 start=True, stop=True)

            # ---- phase 2: per subtile, seed psum with the residual via an
            # identity matmul, accumulate the sampling matmuls on top, then
            # one plain psum->sbuf copy (alternating DVE/ACT).  Blocks of
            # OUTBLK columns go out on the sync HWDGE ring (it is done
            # issuing inputs by then; the scalar ring stays free for
            # copies) as they finish.
            sti = 0
            for b0, bw, sts in blocks:
                for oc in range(2):
                    ob = obp.tile([128, OUTBLK], BF16, tag=f"ob_{oc}_{b0}",
                                  name=f"ob_{oc}_{b0}")
                    for (g, c0, tw) in sts:
                        sti += 1
                        ps = psp.tile([128, SUBTILE], F32, tag="ps",
                                      name=f"ps2_{oc}_{c0}")
                        nc.tensor.matmul(
                            ps[:, 0:tw],
                            lhsT=const_sb[0][:, rkp + C:rkp + C + 128],
                            rhs=inst_sb[oc][:, c0:c0 + tw],
                            start=True, stop=False,
                        )
                        mms = [(sp[g][ch][0], sp[g][ch][1], ch)
                               for ch in range(kch) if sp[g][ch][1]]
                        for i, (t, pw, ch) in enumerate(mms):
                            nc.tensor.matmul(
                                ps[:, 0:tw],
                                lhsT=t[0:pw, oc * 128:(oc + 1) * 128],
                                rhs=wb_sb[ch][0:pw, c0:c0 + tw],
                                start=False,
                                stop=(i == len(mms) - 1),
                            )
                        lc = c0 - b0
                        if sti % 2:
                            nc.vector.tensor_copy(ob[:, lc:lc + tw],
                                                  ps[:, 0:tw])
                        else:
                            nc.scalar.copy(ob[:, lc:lc + tw], ps[:, 0:tw])
                    nc.sync.dma_start(
                        out[oc * 128:(oc + 1) * 128, b0:b0 + bw],
                        ob[:, 0:bw],
                    )

    return nc


def _get_program(structure):
    if structure not in _programs:
        nc = _build_program(structure)
        _split_multiwait(nc)
        nc._wsplit_done = True
        _programs[structure] = nc
    return _programs[structure]


# -------------------------------------------------------------- host prep
def _corners(anchor_bn):
    f = np.float32
    ax = anchor_bn[:, 0].astype(f)
    ay = anchor_bn[:, 1].astype(f)
    gx = (ax - f(XMIN)) / f(XMAX - XMIN + EPS) * f(2.0) - f(1.0)
    gy = (ay - f(YMIN)) / f(YMAX - YMIN + EPS) * f(2.0) - f(1.0)
    # module stacks [grid_y, grid_x]: width coord <- gy, height coord <- gx
    ix = (gy + f(1.0)) * f(0.5) * f(W - 1)
    iy = (gx + f(1.0)) * f(0.5) * f(H - 1)
    x0 = np.floor(ix)
    y0 = np.floor(iy)
    x1 = x0 + f(1.0)
    y1 = y0 + f(1.0)
    wx1 = ix - x0
    wx0 = f(1.0) - wx1
    wy1 = iy - y0
    wy0 = f(1.0) - wy1
    out = []
    for xc, yc, w in ((x0, y0, wx0 * wy0), (x1, y0, wx1 * wy0),
                      (x0, y1, wx0 * wy1), (x1, y1, wx1 * wy1)):
        valid = (xc >= 0) & (xc <= W - 1) & (yc >= 0) & (yc <= H - 1)
        xi = np.clip(xc, 0, W - 1).astype(np.int64)
        yi = np.clip(yc, 0, H - 1).astype(np.int64)
        out.append((xi, yi, valid, (w * valid.astype(f)).astype(f)))
    return out, y0


def _host_fallback(instance_feature, anchor, bev_map, W_proj, b_proj):
    """Exact numpy computation; only for pathological inputs whose bbox
    exceeds RK_MAX."""
    f = np.float32
    out = np.empty((B, N, C), f)
    for b in range(B):
        corners, _ = _corners(anchor[b])
        acc = np.zeros((N, C), f)
        fm = bev_map[b].reshape(C, H * W)
        for xi, yi, valid, w in corners:
            g = fm[:, yi * W + xi].T
            acc += g * w[:, None]
        out[b] = acc @ W_proj.T.astype(f) + b_proj.astype(f)
    return out + instance_feature.astype(f)


# ------------------------------------------------------------------- kernel
def kernel(instance_feature, anchor, anchor_embed, bev_map, W_proj, b_proj):
    global LAST_RESULT
    f = np.float32
    instance_feature = np.asarray(instance_feature)
    anchor = np.asarray(anchor)
    bev_map = np.asarray(bev_map)
    W_proj = np.asarray(W_proj)
    b_proj = np.asarray(b_proj)

    instb = instance_feature.astype(f) + b_proj.astype(f)[None, None, :]

    # ---- pass 1: per-core corner geometry
    cores = []
    for core in range(NCORES):
        b, half = core // 2, core % 2
        sl = slice(half * NPC, (half + 1) * NPC)
        corners, y0f = _corners(anchor[b, sl])
        vx = np.concatenate([np.where(v, xi, -1) for xi, yi, v, w in corners])
        vy = np.concatenate([np.where(v, yi, -1) for xi, yi, v, w in corners])
        m = vx >= 0
        if m.any():
            xmin, xmax = int(vx[m].min()), int(vx[m].max())
            ymin, ymax = int(vy[m].min()), int(vy[m].max())
        else:
            xmin = xmax = ymin = ymax = 0
        if (ymax - ymin + 1) * (xmax - xmin + 1) > RK_MAX:
            return _host_fallback(instance_feature, anchor, bev_map,
                                  W_proj, b_proj)
        cores.append((corners, y0f, xmin, xmax, ymin, ymax))

    # ---- unified structure: GLOBAL row origin so core layouts align
    ymin_g = min(c[4] for c in cores)
    ymax_g = max(c[5] for c in cores)
    Rg = ymax_g - ymin_g + 1
    Kw = max(c[3] - c[2] + 1 for c in cores)
    rpw = max(2, min(128 // max(Kw, 1), Rg)) if Kw <= 64 else 2
    stride = rpw - 1
    n_groups = max(Rg - 2, 0) // stride + 1
    ws = rpw * Kw
    kch = -(-ws // 128)
    rkp = 128 * -(-max(Rg * Kw, (n_groups - 1) * stride * Kw + ws) // 128)
    if rkp > RK_MAX:
        return _host_fallback(instance_feature, anchor, bev_map,
                              W_proj, b_proj)

    y0ps, gs = [], []
    counts = np.zeros((NCORES, n_groups), np.int64)
    for core, (corners, y0f, xmin, xmax, ymin, ymax) in enumerate(cores):
        y0p = np.clip(y0f.astype(np.int64) - ymin_g, 0, max(Rg - 2, 0))
        grp = np.minimum(y0p // stride, n_groups - 1)
        y0ps.append(y0p)
        gs.append(grp)
        counts[core] = np.bincount(grp, minlength=n_groups)
    cap = counts.max(axis=0)

    subtiles = []
    c0 = 0
    for g in range(n_groups):
        left = int(cap[g])
        while left > 0:
            tw = min(SUBTILE, left)
            subtiles.append((g, c0, tw))
            c0 += tw
            left -= tw
    nslot = c0
    structure = (rkp, Kw, ws, stride, kch, n_groups, nslot, tuple(subtiles))

    # ---- pass 2: per-core arrays against the unified layout
    row_base = {}
    base = 0
    for g in range(n_groups):
        row_base[g] = base
        base += int(cap[g])

    maps, perms = [], []
    cw = rkp + C + 128
    wpt = np.ascontiguousarray(W_proj.astype(f).T).astype(NPBF16)
    for core, (corners, y0f, xmin, xmax, ymin, ymax) in enumerate(cores):
        b, half = core // 2, core % 2
        sl = slice(half * NPC, (half + 1) * NPC)
        grp = gs[core]
        # stable sort by group; columns are packed at each group's base
        order = np.argsort(grp, kind="stable")
        cnt = counts[core]
        col_of = np.empty(NPC, np.int64)
        start = 0
        for g in range(n_groups):
            end = start + int(cnt[g])
            col_of[order[start:end]] = row_base[g] + np.arange(end - start)
            start = end

        consts = np.zeros((C, cw), NPBF16)
        ke = min(xmin + Kw, W)
        ye = min(ymin_g + Rg, H)
        bev_rows = bev_map[b][:, ymin_g:ye, xmin:ke].astype(f)
        tmp = np.zeros((C, Rg, Kw), f)
        tmp[:, :ye - ymin_g, :ke - xmin] = bev_rows
        consts[:, :Rg * Kw] = tmp.reshape(C, Rg * Kw).astype(NPBF16)
        consts[:, rkp:rkp + C] = wpt
        consts[:128, rkp + C:rkp + C + 128] = np.eye(128, dtype=NPBF16)

        wb = np.zeros((kch * 128, nslot), NPBF16)
        for xi, yi, valid, wgt in corners:
            px = (yi - ymin_g - grp * stride) * Kw + (xi - xmin)
            wb[px[valid], col_of[valid]] = wgt[valid].astype(NPBF16)

        instb_t = np.zeros((C, nslot), NPBF16)
        instb_t[:, col_of] = instb[b, sl].T.astype(NPBF16)

        maps.append({
            "consts": consts,
            "wb": wb,
            "instb_t": instb_t,
        })
        perms.append(col_of)

    nc = _get_program(structure)
    res = run_bass_kernel_spmd(nc, maps, list(range(NCORES)), trace=TRACE)
    LAST_RESULT = res

    out = np.empty((B, N, C), f)
    for core in range(NCORES):
        b, half = core // 2, core % 2
        sl = slice(half * NPC, (half + 1) * NPC)
        o = res.results[core]["out_t"]
        out[b, sl] = o[:, perms[core]].T.astype(f)
    return out
